# revision 1
# baseline (speedup 1.0000x reference)
"""Distributed multi-head attention kernel for 8 TRN2 NeuronCores.

Module: B=2, N=2048, D_MODEL=1024, H=16, D_HEAD=64 attention with
arbitrary rotary embedding, key-side boolean masking, softmax, and
output projection.

Sharding: head-parallel attention (2 heads per core, both batches),
then one AllToAll (~1 MB/core, bf16) to switch to row-parallel for the
output projection. Each core returns a [512, 1024] row block.

Key design points:
 - All matmuls bf16 with fp32 PSUM accumulation (bf16 lets the PE
   pipeline LDWEIGHTS; fp32/f32r serialize it). End-to-end ~5e-3 rel.
 - qT/kT produced in [chan, row] layout so scores come out transposed
   [keys, qrows] with keys on partitions.
 - Rotary via host-rotated weight copies: rot2(x@W) == x@Wr.
 - Key mask folded into the softmax exp as a per-partition bias.
 - Softmax denominator from a ones-column in V (lhsT = [v | 1], M=65);
   normalization happens after the AllToAll (denominators travel in
   the same buffer: shard layout [hA(64) | denA | hB(64) | denB]).
 - Scores for the two heads issue to PE row groups (0,0)/(64,0) so
   they execute concurrently.
 - One start=True per PSUM bank per accumulation chain (start clears
   the whole bank).
 - DMAs are spread across both HWDGE queues (SP + ACT) and ordered so
   the first projection matmuls start ~10us in.
"""
import os
import warnings

warnings.filterwarnings("ignore")
import numpy as np
import ml_dtypes

from concourse import bacc, tile, mybir, bass_utils

B, N, DM, H, DH = 2, 2048, 1024, 16, 64
R = B * N
NCORES = 8
HPC = 2
CPC = HPC * DH       # 128 chans per core
KT = 8               # contraction tiles over d_model
RB = 8               # row blocks of 512 over R
NKEYT = 16           # key tiles of 128 over N
ROWS_PER_CORE = R // NCORES  # 512
QHS = 1024           # qrows per phase-2 inner pass

F32 = mybir.dt.float32
BF16 = mybir.dt.bfloat16

SHARD_ROWS = CPC + HPC  # 130: [hA 64 | denA 1 | hB 64 | denB 1]

LAST_EXEC_TIME_NS = None
LAST_TRACE_DIR = None


def _install_trace_shim():
    import sys
    import types
    import ctypes
    import contextlib

    if "antenv.axon_hooks" in sys.modules:
        return
    so_path = "/opt/axon/libaxon_pjrt.so"
    hook = None
    if os.path.exists(so_path):
        lib = ctypes.CDLL(so_path)
        if hasattr(lib, "axon_start_nrt_profile"):
            lib.axon_start_nrt_profile.argtypes = [
                ctypes.POINTER(ctypes.c_int64), ctypes.c_size_t]
            lib.axon_start_nrt_profile.restype = ctypes.c_int64
            lib.axon_stop_nrt_profile.argtypes = [ctypes.c_char_p]
            lib.axon_stop_nrt_profile.restype = ctypes.c_int64

            @contextlib.contextmanager
            def _hook(output_dir, device_ids):
                import jax
                jax.devices()
                if device_ids:
                    ids = (ctypes.c_int64 * len(device_ids))(*device_ids)
                    rc = lib.axon_start_nrt_profile(ids, len(device_ids))
                else:
                    rc = lib.axon_start_nrt_profile(None, 0)
                if rc != 0:
                    raise RuntimeError(f"axon_start_nrt_profile rc={rc}")
                try:
                    yield
                finally:
                    n = lib.axon_stop_nrt_profile(str(output_dir).encode())
                    print(f"[trace] {n} profile file(s) -> {output_dir}")

            hook = _hook

    mod = types.ModuleType("antenv.axon_hooks")
    mod.get_axon_ntff_profile_hook = lambda: hook
    mod.set_axon_ntff_profile_hook = lambda h: None
    sys.modules["antenv.axon_hooks"] = mod
    bass_utils.upload_artifacts = lambda tmpdir: tmpdir


_LDW_PATCHED = False


def _enable_ldw_opt():
    """concourse hardcodes --enable-ldw-opt=false; with it off every
    LDWEIGHTS serializes with its matmul (~+115ns/MM). Flip it on."""
    global _LDW_PATCHED
    if _LDW_PATCHED:
        return
    _LDW_PATCHED = True
    orig = bass_utils.run_command

    def patched(cmd, *a, **kw):
        if isinstance(cmd, list):
            cmd = ["--enable-ldw-opt=true" if c == "--enable-ldw-opt=false"
                   else c for c in cmd]
        return orig(cmd, *a, **kw)

    bass_utils.run_command = patched


def _rot_cols(w):
    wr = np.empty_like(w)
    wr[:, 0::2] = -w[:, 1::2]
    wr[:, 1::2] = w[:, 0::2]
    return wr


def build(dbg=False):
    nc = bacc.Bacc("TRN2", target_bir_lowering=False, debug=False,
                   num_devices=NCORES)

    xt_d = nc.dram_tensor("xt", [DM, R], BF16, kind="ExternalInput")
    wq_d = nc.dram_tensor("wq", [DM, CPC], BF16, kind="ExternalInput")
    wqr_d = nc.dram_tensor("wqr", [DM, CPC], BF16, kind="ExternalInput")
    wk_d = nc.dram_tensor("wk", [DM, CPC], BF16, kind="ExternalInput")
    wkr_d = nc.dram_tensor("wkr", [DM, CPC], BF16, kind="ExternalInput")
    wv_d = nc.dram_tensor("wv", [DM, CPC], BF16, kind="ExternalInput")
    wout_d = nc.dram_tensor("wout", [DM, DM], BF16, kind="ExternalInput")
    boutb_d = nc.dram_tensor("boutb", [128, DM], F32, kind="ExternalInput")
    cost_d = nc.dram_tensor("cost", [CPC, N], BF16, kind="ExternalInput")
    sint_d = nc.dram_tensor("sint", [CPC, N], BF16, kind="ExternalInput")
    maskb_d = nc.dram_tensor("maskb", [128, R // 128], F32, kind="ExternalInput")
    vones_d = nc.dram_tensor("vones", [128, (R // 128) * 2], BF16,
                             kind="ExternalInput")
    selb_d = nc.dram_tensor("selb", [128, 1], F32, kind="ExternalInput")
    selab_d = nc.dram_tensor("selab", [2, 128], BF16, kind="ExternalInput")
    selbi_d = nc.dram_tensor("selbi", [128, 1], F32, kind="ExternalInput")

    out_d = nc.dram_tensor("out", [ROWS_PER_CORE, DM], F32, kind="ExternalOutput")

    a2a_in = [nc.dram_tensor(f"a2a_in{b}", [NCORES * SHARD_ROWS, ROWS_PER_CORE],
                             BF16) for b in range(B)]
    a2a_out = [nc.dram_tensor(f"a2a_out{b}", [NCORES * SHARD_ROWS, ROWS_PER_CORE],
                              BF16) for b in range(B)]

    VAUGW = 2 * (DH + 1)  # 130 cols per key tile: [vA | 1 | vB | 1]

    with tile.TileContext(nc) as tc:
        with tc.tile_pool(name="persist", bufs=1) as pp:
            wq_sb = pp.tile([128, KT, CPC], BF16, tag="wq")
            wqr_sb = pp.tile([128, KT, CPC], BF16, tag="wqr")
            wk_sb = pp.tile([128, KT, CPC], BF16, tag="wk")
            wkr_sb = pp.tile([128, KT, CPC], BF16, tag="wkr")
            wv_sb = pp.tile([128, KT, CPC], BF16, tag="wv")
            cost_sb = pp.tile([CPC, N], BF16, tag="cost")
            sint_sb = pp.tile([CPC, N], BF16, tag="sint")
            maskb_sb = pp.tile([128, R // 128], F32, tag="maskb")
            boutb_sb = pp.tile([128, DM], F32, tag="boutb")
            qt_sb = pp.tile([CPC, R], BF16, tag="qt")
            kt_sb = pp.tile([CPC, R], BF16, tag="kt")
            vaug_sb = pp.tile([128, (R // 128) * VAUGW], BF16, tag="vaug")
            wo_sb = pp.tile([128, KT, DM], BF16, tag="wo")

            def ktview(d):
                return d.ap().rearrange("(k p) n -> p k n", p=128)

            xt_view = xt_d.ap().rearrange("(k p) n -> p k n", p=128)

            # first xt block + weights first so matmuls start early;
            # per-kt pieces across both queues so matmul #0 only waits for
            # its own contraction slice
            xt_sb0 = pp.tile([128, KT, 512], BF16, tag="xt0")
            for kt in range(KT):
                eng = nc.sync if kt % 2 == 0 else nc.scalar
                eng.dma_start(xt_sb0[:, kt, :], xt_view[:, kt, 0:512])
            nc.sync.dma_start(wq_sb[:], ktview(wq_d))
            nc.scalar.dma_start(wqr_sb[:], ktview(wqr_d))
            nc.sync.dma_start(wk_sb[:], ktview(wk_d))
            nc.scalar.dma_start(wkr_sb[:], ktview(wkr_d))
            nc.sync.dma_start(wv_sb[:], ktview(wv_d))
            # pre-load the ACT Exp table during the initial DMA wait so the
            # first real softmax exp doesn't stall the pipeline (a PE idle
            # gap there re-throttles the HAM clock gate)
            warm_sb = pp.tile([1, 2], F32, tag="warm")
            nc.vector.memset(warm_sb[:], 0.0)
            nc.scalar.activation(warm_sb[0:1, 1:2], warm_sb[0:1, 0:1],
                                 mybir.ActivationFunctionType.Exp)
            nc.scalar.dma_start(cost_sb[:], cost_d[:, :])
            nc.scalar.dma_start(sint_sb[:], sint_d[:, :])
            nc.scalar.dma_start(maskb_sb[:], maskb_d[:, :])
            ones_view = vaug_sb[:].rearrange("p (t u w) -> p (t u) w",
                                             u=2, w=DH + 1)[:, :, DH]
            nc.scalar.dma_start(ones_view, vones_d[:, :])
            selb_sb = pp.tile([128, 1], F32, tag="selb")
            selab0_sb = pp.tile([1, 128], BF16, tag="selab0")
            selab1_sb = pp.tile([1, 128], BF16, tag="selab1")
            selbi_sb = pp.tile([128, 1], F32, tag="selbi")

            # ---- Phase 1: projections + rotary + v_aug ----
            with tc.tile_pool(name="p1", bufs=2) as p1, \
                 tc.tile_pool(name="ps1", bufs=1, space="PSUM") as ps1:
                for rb in range(RB):
                    c0 = rb * 512
                    if rb == 0:
                        xt_sb = xt_sb0
                    else:
                        xt_sb = p1.tile([128, KT, 512], BF16, tag="xt")
                        if rb == 4:
                            eng = nc.gpsimd
                        elif rb % 2 == 1:
                            eng = nc.sync
                        else:
                            eng = nc.scalar
                        eng.dma_start(xt_sb[:], xt_view[:, :, c0:c0 + 512])

                    q_ps = ps1.tile([128, 512], F32, tag="q")
                    qr_ps = ps1.tile([128, 512], F32, tag="qr")
                    k_ps = ps1.tile([128, 512], F32, tag="k")
                    kr_ps = ps1.tile([128, 512], F32, tag="kr")
                    v_ps = ps1.tile([128, 512], F32, tag="v")
                    for kt in range(KT):
                        st, sp = kt == 0, kt == KT - 1
                        for ps_t, w_t in [(q_ps, wq_sb), (qr_ps, wqr_sb),
                                          (k_ps, wk_sb), (kr_ps, wkr_sb)]:
                            nc.tensor.matmul(ps_t[:], w_t[:, kt, :],
                                             xt_sb[:, kt, :], start=st, stop=sp)
                        for vt in range(4):
                            nc.tensor.matmul(
                                v_ps[:, vt * 128:(vt + 1) * 128],
                                xt_sb[:, kt, vt * 128:(vt + 1) * 128],
                                wv_sb[:, kt, :], start=(st and vt == 0), stop=sp)

                    cc = c0 % N
                    tmp = p1.tile([128, 512], BF16, tag="rottmp")
                    for dst, a_ps, b_ps in [(qt_sb, q_ps, qr_ps),
                                            (kt_sb, k_ps, kr_ps)]:
                        dv = dst[:, c0:c0 + 512]
                        nc.vector.tensor_mul(dv, a_ps[:], cost_sb[:, cc:cc + 512])
                        nc.vector.tensor_mul(tmp[:], b_ps[:], sint_sb[:, cc:cc + 512])
                        nc.vector.tensor_add(dv, dv, tmp[:])

                    kt0 = rb * 4
                    va = vaug_sb[:].rearrange("p (t w) -> p t w", w=VAUGW)
                    vp = v_ps[:].rearrange("p (t c) -> p t c", c=128)
                    nc.vector.tensor_copy(va[:, kt0:kt0 + 4, 0:DH],
                                          vp[:, :, 0:DH])
                    nc.vector.tensor_copy(va[:, kt0:kt0 + 4, DH + 1:DH + 1 + DH],
                                          vp[:, :, DH:2 * DH])

                # keep PE busy across the phase transition (an idle gap
                # here re-throttles the PE clock for the rest of the run)
                brid_ps = ps1.tile([128, 512], F32, tag="brid")
                for i in range(12):
                    nc.tensor.matmul(brid_ps[:], wq_sb[:, i % KT, :],
                                     xt_sb0[:, i % KT, :],
                                     start=(i == 0), stop=(i == 11))

            # wout needed only in phase 3 — load it behind phase-1 traffic
            nc.scalar.dma_start(wo_sb[:], wout_d.ap().rearrange(
                "(k p) n -> p k n", p=128))
            nc.sync.dma_start(boutb_sb[:], boutb_d[:, :])
            nc.sync.dma_start(selb_sb[:], selb_d[:, :])
            nc.sync.dma_start(selab0_sb[:], selab_d[0:1, :])
            nc.sync.dma_start(selab1_sb[:], selab_d[1:2, :])
            nc.sync.dma_start(selbi_sb[:], selbi_d[:, :])
            # zero the shard halves each batch's A2A never writes (their
            # content hits the sel blend; garbage could be Inf/NaN)
            zt = pp.tile([128, 512], BF16, tag="zt")
            nc.vector.memset(zt[:], 0.0)
            for b in range(B):
                for j in range(NCORES):
                    if (j // 4) != b:
                        r0 = j * SHARD_ROWS
                        nc.sync.dma_start(a2a_in[b][r0:r0 + 128, :], zt[:])
                        nc.sync.dma_start(a2a_in[b][r0 + 128:r0 + SHARD_ROWS, :],
                                          zt[0:2, :])

            # ---- Phase 2: attention, two heads packed, per (b, q-half) ----
            with tc.tile_pool(name="p2", bufs=2) as p2, \
                 tc.tile_pool(name="ps_sc", bufs=1, space="PSUM") as ps_sc, \
                 tc.tile_pool(name="ps_o", bufs=1, space="PSUM") as ps_o:
                for b in range(B):
                    for qh in range(N // QHS):
                        qbase = b * N + qh * QHS
                        o_ps = [ps_o.tile([DH + 1, QHS], F32, tag=f"outp{h}",
                                          name=f"ops{h}") for h in range(HPC)]
                        for kt in range(NKEYT):
                            g = b * NKEYT + kt
                            krow = b * N + kt * 128
                            sc = [ps_sc.tile([128, QHS], F32, tag=f"sc{h}",
                                        name=f"sc{h}") for h in range(HPC)]
                            # interleave heads so the PE runs them in
                            # different row groups concurrently
                            for qq in range(QHS // 512):
                                for h in range(HPC):
                                    ho = h * DH
                                    nc.tensor.matmul(
                                        sc[h][:, qq * 512:(qq + 1) * 512],
                                        kt_sb[ho:ho + DH, krow:krow + 128],
                                        qt_sb[ho:ho + DH,
                                              qbase + qq * 512:qbase + (qq + 1) * 512],
                                        start=True, stop=True)
                            p_sb = []
                            for h in range(HPC):
                                pt = p2.tile([128, QHS], BF16, tag=f"p{h}",
                                             name=f"pt{h}")
                                nc.scalar.activation(
                                    pt[:], sc[h][:],
                                    mybir.ActivationFunctionType.Exp,
                                    bias=maskb_sb[:, g:g + 1],
                                    scale=float(DH ** -0.5))
                                p_sb.append(pt)
                            for h in range(HPC):
                                va_l = vaug_sb[:, g * VAUGW + h * (DH + 1):
                                               g * VAUGW + (h + 1) * (DH + 1)]
                                for qq in range(QHS // 512):
                                    nc.tensor.matmul(
                                        o_ps[h][:, qq * 512:(qq + 1) * 512],
                                        va_l,
                                        p_sb[h][:, qq * 512:(qq + 1) * 512],
                                        start=(kt == 0), stop=(kt == NKEYT - 1))

                        # tail: one bf16 copy + two [65, 512] DMAs per head
                        for h in range(HPC):
                            onb = p2.tile([DH + 1, QHS], BF16, tag=f"onb{h}",
                                          name=f"onb{h}")
                            nc.vector.tensor_copy(onb[:], o_ps[h][:])
                            for u in range(QHS // 512):
                                j = b * 4 + qh * (QHS // 512) + u
                                r0 = j * SHARD_ROWS + h * (DH + 1)
                                nc.sync.dma_start(
                                    a2a_in[b][r0: r0 + DH + 1, :],
                                    onb[:, u * 512:(u + 1) * 512])
                    if qh == N // QHS - 1:
                        nc.gpsimd.collective_compute(
                            "AllToAll", mybir.AluOpType.bypass,
                            replica_groups=[list(range(NCORES))],
                            ins=[a2a_in[b].ap().opt()],
                            outs=[a2a_out[b].ap().opt()])


            with tc.tile_pool(name="p3", bufs=1) as p3, \
                 tc.tile_pool(name="p3b", bufs=2) as p3b, \
                 tc.tile_pool(name="ps3", bufs=2, space="PSUM") as ps3:
                ob = []
                dn = []
                for b in range(B):
                    o_t = p3.tile([128, KT, 512], BF16, tag=f"oallb{b}",
                                  name=f"oallb{b}")
                    d_t = p3.tile([2 * NCORES, 512], BF16, tag=f"denb{b}",
                                  name=f"denb{b}")
                    av = a2a_out[b].ap().rearrange("(j q) n -> q j n",
                                                   q=SHARD_ROWS)
                    nc.sync.dma_start(d_t[0:NCORES, :], av[DH:DH + 1, :, :])
                    nc.sync.dma_start(d_t[NCORES:2 * NCORES, :],
                                      av[CPC + 1:CPC + 2, :, :])
                    nc.sync.dma_start(o_t[0:DH, :, :], av[0:DH, :, :])
                    nc.scalar.dma_start(o_t[DH:CPC, :, :], av[DH + 1:CPC + 1, :, :])
                    ob.append(o_t)
                    dn.append(d_t)
                # blend mine = b0*sel + b1*(1-sel); the b0 terms compute as
                # soon as A2A#0 lands (hidden under batch-1 attention)
                oall_sb = p3.tile([128, KT, 512], BF16, tag="oall")
                t1_sb = p3.tile([128, KT, 512], BF16, tag="t1")
                nc.vector.tensor_scalar_mul(oall_sb[:], ob[0][:], selb_sb[:])
                nc.vector.tensor_scalar_mul(t1_sb[:], ob[1][:], selbi_sb[:])
                nc.vector.tensor_add(oall_sb[:], oall_sb[:], t1_sb[:])
                den_sb = p3.tile([2 * NCORES, 512], BF16, tag="den")
                dt1_sb = p3.tile([2 * NCORES, 512], BF16, tag="dt1")
                nc.vector.tensor_scalar_mul(den_sb[:], dn[0][:],
                                            selb_sb[0:2 * NCORES, :])
                nc.vector.tensor_scalar_mul(dt1_sb[:], dn[1][:],
                                            selbi_sb[0:2 * NCORES, :])
                nc.vector.tensor_add(den_sb[:], den_sb[:], dt1_sb[:])
                recip_sb = p3.tile([2 * NCORES, 512], F32, tag="recip")
                nc.vector.reciprocal(recip_sb[:], den_sb[:])
                # gather recips into partition 0, then broadcast to all 128
                rrow = p3.tile([1, 2 * NCORES * 512], F32, tag="rrow")
                nc.sync.dma_start(
                    rrow[:].rearrange("p (u n) -> p u n", n=512), recip_sb[:])
                divall = p3.tile([128, 2 * NCORES * 512], F32, tag="divall")
                nc.gpsimd.partition_broadcast(divall[:], rrow[:])

                onorm_sb = p3.tile([128, KT, 512], BF16, tag="onorm")
                for kt in range(KT):
                    nc.vector.tensor_mul(
                        onorm_sb[0:DH, kt, :], oall_sb[0:DH, kt, :],
                        divall[0:DH, kt * 512:(kt + 1) * 512])
                    nc.vector.tensor_mul(
                        onorm_sb[DH:CPC, kt, :], oall_sb[DH:CPC, kt, :],
                        divall[DH:CPC, (NCORES + kt) * 512:(NCORES + kt + 1) * 512])

                for rw in range(4):
                    y_ps = ps3.tile([128, DM], F32, tag="y")
                    for kt in range(KT):
                        st, sp = kt == 0, kt == KT - 1
                        for nb in range(2):
                            nc.tensor.matmul(
                                y_ps[:, nb * 512:(nb + 1) * 512],
                                onorm_sb[:, kt, rw * 128:(rw + 1) * 128],
                                wo_sb[:, kt, nb * 512:(nb + 1) * 512],
                                start=st, stop=sp)
                    y_sb = p3b.tile([128, DM], F32, tag="y_sb")
                    nc.vector.tensor_add(y_sb[:], y_ps[:], boutb_sb[:])
                    eng = nc.sync if rw % 2 == 0 else nc.scalar
                    eng.dma_start(out_d[rw * 128:(rw + 1) * 128, :], y_sb[:])

    nc.compile()
    return nc


_NC_CACHE = None


def kernel(x, mask, pos_emb, Wq, Wkv, Wout, bout):
    global LAST_EXEC_TIME_NS, LAST_TRACE_DIR, _NC_CACHE

    x = np.asarray(x, dtype=np.float32)
    mask = np.asarray(mask)
    pos_emb = np.asarray(pos_emb, dtype=np.float32)
    Wq = np.asarray(Wq, dtype=np.float32)
    Wkv = np.asarray(Wkv, dtype=np.float32)
    Wout = np.asarray(Wout, dtype=np.float32)
    bout = np.asarray(bout, dtype=np.float32)

    bf = ml_dtypes.bfloat16
    xt = np.ascontiguousarray(x.reshape(R, DM).T).astype(bf)
    wk_full = Wkv[:, :H * DH]
    wv_full = Wkv[:, H * DH:]
    cost = np.ascontiguousarray(np.tile(np.cos(pos_emb).T, (HPC, 1))).astype(bf)
    sint = np.ascontiguousarray(np.tile(np.sin(pos_emb).T, (HPC, 1))).astype(bf)
    maskb = np.ascontiguousarray(
        np.where(mask.reshape(R), 0.0, -1e5).astype(np.float32)
        .reshape(R // 128, 128).T)
    boutb = np.ascontiguousarray(
        np.broadcast_to(bout[None, :], (128, DM)).astype(np.float32))
    wqr = _rot_cols(Wq)
    wkr = _rot_cols(wk_full)
    selab = np.zeros((2, 128), dtype=bf)
    selab[0, 0:DH] = 1.0
    selab[1, DH:CPC] = 1.0

    in_maps = []
    for c in range(NCORES):
        cols = slice(c * CPC, (c + 1) * CPC)
        in_maps.append({
            "xt": xt,
            "wq": np.ascontiguousarray(Wq[:, cols]).astype(bf),
            "wqr": np.ascontiguousarray(wqr[:, cols]).astype(bf),
            "wk": np.ascontiguousarray(wk_full[:, cols]).astype(bf),
            "wkr": np.ascontiguousarray(wkr[:, cols]).astype(bf),
            "wv": np.ascontiguousarray(wv_full[:, cols]).astype(bf),
            "wout": Wout.astype(bf),
            "boutb": boutb,
            "cost": cost,
            "sint": sint,
            "maskb": maskb,
            "vones": np.ones((128, (R // 128) * 2), dtype=bf),
            "selb": np.full((128, 1), 1.0 if c < 4 else 0.0, dtype=np.float32),
            "selab": selab,
            "selbi": np.full((128, 1), 0.0 if c < 4 else 1.0, dtype=np.float32),
        })

    # walrus rejects bass-emitted InstLdweights under ldw-opt; keep off
    if bool(int(os.environ.get("BASS_LDW_OPT", "0"))):
        _enable_ldw_opt()
    dbg = bool(int(os.environ.get("BASS_KERNEL_DEBUG", "0")))
    if _NC_CACHE is None:
        _NC_CACHE = build(dbg=dbg)
    nc = _NC_CACHE

    trace = bool(int(os.environ.get("BASS_KERNEL_TRACE", "0")))
    kwargs = {}
    if trace:
        _install_trace_shim()
        tdir = os.environ.get("BASS_TRACE_DIR", "/tmp/bass_trace_out")
        os.makedirs(tdir, exist_ok=True)
        kwargs["tmpdir"] = tdir
    res = bass_utils.run_bass_kernel_spmd(
        nc, in_maps, core_ids=list(range(NCORES)), trace=trace, **kwargs)
    LAST_EXEC_TIME_NS = res.exec_time_ns
    if res.instructions_and_trace is not None:
        LAST_TRACE_DIR = res.instructions_and_trace[1]
        globals()["LAST_INSTS"] = res.instructions_and_trace[0]

    globals()["LAST_RESULTS"] = res.results
    y = np.concatenate([res.results[c]["out"] for c in range(NCORES)], axis=0)
    return y.reshape(B, N, DM)



# revision 3
# speedup vs baseline: 1.3425x; 1.3425x over previous
"""Distributed multi-head attention kernel for 8 TRN2 NeuronCores.

Module: B=2, N=2048, D_MODEL=1024, H=16, D_HEAD=64 attention with
arbitrary rotary embedding, key-side boolean masking, softmax, and
output projection.

Sharding: head-parallel attention (2 heads per core, both batches),
then one AllToAll per batch (~1 MB/core, bf16) to switch to
row-parallel for the output projection. Each core returns a
[512, 1024] row block.

v2 restructure (vs the 478-538us baseline):
 - Attention emitted software-pipelined per 512-q-row pass: the two
   heads' [128,512] score blocks share one [128,1024] PSUM tile
   (adjacent banks), one exp per key tile covers both heads, score
   MMs for key tile kt+1 are emitted before attnV of kt so the PE
   never waits on the scalar engine.  Score PSUM is triple-buffered
   (6 banks) + 2 o-accumulator banks = 8.  This keeps the PE busy
   enough that the HAM clock gate stays at 8/8 (the baseline ran the
   whole attention phase at 4/8 = 1.2 GHz).
 - Softmax denominators via a ones-column in V (lhsT = [v | 1], M=65);
   key mask folded into the exp as a per-partition bias.
 - Phase 3 split per batch: batch-0's blend/normalize/projection runs
   right after AllToAll#1 is issued (hiding the collective), batch-1's
   at the tail.  Since A2A#b carries zeros in the shards destined for
   the other half's cores, y(b) is exact on the cores that need it and
   zero elsewhere: out = y0(+bias) + y1.
 - Softmax normalization: 1/den via reciprocal_approx_fast, broadcast
   to 128 partitions with one selector matmul per core-slot (lhsT is a
   host-built [16,128] 0/1 matrix) instead of the 37us gpsimd
   partition_broadcast.
 - All matmuls bf16 with fp32 PSUM accumulation; rotary via
   host-rotated weight copies: rot2(x@W) == x@Wr.
"""
import os
import warnings

warnings.filterwarnings("ignore")
import numpy as np
import ml_dtypes

from concourse import bacc, tile, mybir, bass_utils

B, N, DM, H, DH = 2, 2048, 1024, 16, 64
R = B * N
NCORES = 8
HPC = 2
CPC = HPC * DH       # 128 chans per core
KT = 8               # contraction tiles over d_model
RB = 8               # row blocks of 512 over R
NKEYT = 16           # key tiles of 128 over N
ROWS_PER_CORE = R // NCORES  # 512
QC = 512             # q rows per attention pass
NPASS = N // QC      # 4 passes per batch

F32 = mybir.dt.float32
BF16 = mybir.dt.bfloat16

SHARD_ROWS = CPC + HPC  # 130: [hA 64 | denA 1 | hB 64 | denB 1]
VAUGW = 2 * (DH + 1)    # 130 cols per key tile: [vA | 1 | vB | 1]

LAST_EXEC_TIME_NS = None
LAST_TRACE_DIR = None


def _install_trace_shim():
    import sys
    import types
    import ctypes
    import contextlib

    if "antenv.axon_hooks" in sys.modules:
        return
    so_path = "/opt/axon/libaxon_pjrt.so"
    hook = None
    if os.path.exists(so_path):
        lib = ctypes.CDLL(so_path)
        if hasattr(lib, "axon_start_nrt_profile"):
            lib.axon_start_nrt_profile.argtypes = [
                ctypes.POINTER(ctypes.c_int64), ctypes.c_size_t]
            lib.axon_start_nrt_profile.restype = ctypes.c_int64
            lib.axon_stop_nrt_profile.argtypes = [ctypes.c_char_p]
            lib.axon_stop_nrt_profile.restype = ctypes.c_int64

            @contextlib.contextmanager
            def _hook(output_dir, device_ids):
                import jax
                jax.devices()
                if device_ids:
                    ids = (ctypes.c_int64 * len(device_ids))(*device_ids)
                    rc = lib.axon_start_nrt_profile(ids, len(device_ids))
                else:
                    rc = lib.axon_start_nrt_profile(None, 0)
                if rc != 0:
                    raise RuntimeError(f"axon_start_nrt_profile rc={rc}")
                try:
                    yield
                finally:
                    n = lib.axon_stop_nrt_profile(str(output_dir).encode())
                    print(f"[trace] {n} profile file(s) -> {output_dir}")

            hook = _hook

    mod = types.ModuleType("antenv.axon_hooks")
    mod.get_axon_ntff_profile_hook = lambda: hook
    mod.set_axon_ntff_profile_hook = lambda h: None
    sys.modules["antenv.axon_hooks"] = mod
    bass_utils.upload_artifacts = lambda tmpdir: tmpdir


def _rot_cols(w):
    wr = np.empty_like(w)
    wr[:, 0::2] = -w[:, 1::2]
    wr[:, 1::2] = w[:, 0::2]
    return wr


def build(dbg=False):
    nc = bacc.Bacc("TRN2", target_bir_lowering=False, debug=False,
                   num_devices=NCORES)

    xt_d = nc.dram_tensor("xt", [DM, R], BF16, kind="ExternalInput")
    wq_d = nc.dram_tensor("wq", [DM, CPC], BF16, kind="ExternalInput")
    wqr_d = nc.dram_tensor("wqr", [DM, CPC], BF16, kind="ExternalInput")
    wk_d = nc.dram_tensor("wk", [DM, CPC], BF16, kind="ExternalInput")
    wkr_d = nc.dram_tensor("wkr", [DM, CPC], BF16, kind="ExternalInput")
    wv_d = nc.dram_tensor("wv", [DM, CPC], BF16, kind="ExternalInput")
    wout_d = nc.dram_tensor("wout", [DM, DM], BF16, kind="ExternalInput")
    boutb_d = nc.dram_tensor("boutb", [128, DM], F32, kind="ExternalInput")
    cost_d = nc.dram_tensor("cost", [CPC, N], BF16, kind="ExternalInput")
    sint_d = nc.dram_tensor("sint", [CPC, N], BF16, kind="ExternalInput")
    maskb_d = nc.dram_tensor("maskb", [128, R // 128], F32, kind="ExternalInput")
    vones_d = nc.dram_tensor("vones", [128, (R // 128) * 2], BF16,
                             kind="ExternalInput")
    selb_d = nc.dram_tensor("selb", [128, 1], F32, kind="ExternalInput")
    selbi_d = nc.dram_tensor("selbi", [128, 1], F32, kind="ExternalInput")
    sel16_d = nc.dram_tensor("sel16", [16, 8 * 128], BF16, kind="ExternalInput")

    out_d = nc.dram_tensor("out", [ROWS_PER_CORE, DM], F32, kind="ExternalOutput")

    a2a_in = [nc.dram_tensor(f"a2a_in{b}", [NCORES * SHARD_ROWS, ROWS_PER_CORE],
                             BF16) for b in range(B)]
    a2a_out = [nc.dram_tensor(f"a2a_out{b}", [NCORES * SHARD_ROWS, ROWS_PER_CORE],
                              BF16) for b in range(B)]

    with tile.TileContext(nc) as tc:
        with tc.tile_pool(name="persist", bufs=1) as pp:
            wq_sb = pp.tile([128, KT, CPC], BF16, tag="wq")
            wqr_sb = pp.tile([128, KT, CPC], BF16, tag="wqr")
            wk_sb = pp.tile([128, KT, CPC], BF16, tag="wk")
            wkr_sb = pp.tile([128, KT, CPC], BF16, tag="wkr")
            wv_sb = pp.tile([128, KT, CPC], BF16, tag="wv")
            cost_sb = pp.tile([CPC, N], BF16, tag="cost")
            sint_sb = pp.tile([CPC, N], BF16, tag="sint")
            maskb_sb = pp.tile([128, R // 128], F32, tag="maskb")
            boutb_sb = pp.tile([128, DM], F32, tag="boutb")
            qt_sb = pp.tile([CPC, R], BF16, tag="qt")
            kt_sb = pp.tile([CPC, R], BF16, tag="kt")
            vaug_sb = pp.tile([128, (R // 128) * VAUGW], BF16, tag="vaug")
            wo_sb = pp.tile([128, KT, DM], BF16, tag="wo")
            y0_sb = pp.tile([128, 4, DM], F32, tag="y0")

            def ktview(d):
                return d.ap().rearrange("(k p) n -> p k n", p=128)

            xt_view = xt_d.ap().rearrange("(k p) n -> p k n", p=128)

            # first xt block + weights first so matmuls start early;
            # per-kt pieces across both queues so matmul #0 only waits for
            # its own contraction slice
            xt_sb0 = pp.tile([128, KT, 512], BF16, tag="xt0")
            for kt in range(KT):
                eng = nc.sync if kt % 2 == 0 else nc.scalar
                eng.dma_start(xt_sb0[:, kt, :], xt_view[:, kt, 0:512])
            nc.sync.dma_start(wq_sb[:], ktview(wq_d))
            nc.scalar.dma_start(wqr_sb[:], ktview(wqr_d))
            nc.sync.dma_start(wk_sb[:], ktview(wk_d))
            nc.scalar.dma_start(wkr_sb[:], ktview(wkr_d))
            nc.sync.dma_start(wv_sb[:], ktview(wv_d))
            # pre-load the ACT Exp table during the initial DMA wait so the
            # first real softmax exp doesn't stall the pipeline
            warm_sb = pp.tile([1, 2], F32, tag="warm")
            nc.vector.memset(warm_sb[:], 0.0)
            nc.scalar.activation(warm_sb[0:1, 1:2], warm_sb[0:1, 0:1],
                                 mybir.ActivationFunctionType.Exp)
            nc.scalar.dma_start(cost_sb[:], cost_d[:, :])
            nc.scalar.dma_start(sint_sb[:], sint_d[:, :])
            nc.scalar.dma_start(maskb_sb[:], maskb_d[:, :])
            ones_view = vaug_sb[:].rearrange("p (t u w) -> p (t u) w",
                                             u=2, w=DH + 1)[:, :, DH]
            nc.scalar.dma_start(ones_view, vones_d[:, :])
            selb_sb = pp.tile([128, 1], F32, tag="selb")
            selbi_sb = pp.tile([128, 1], F32, tag="selbi")
            sel16_sb = pp.tile([16, 8 * 128], BF16, tag="sel16")

            # ---- Phase 1: projections + rotary + v_aug ----
            with tc.tile_pool(name="p1", bufs=2) as p1, \
                 tc.tile_pool(name="ps1", bufs=1, space="PSUM") as ps1:
                for rb in range(RB):
                    c0 = rb * 512
                    if rb == 0:
                        xt_sb = xt_sb0
                    else:
                        xt_sb = p1.tile([128, KT, 512], BF16, tag="xt")
                        if rb == 4:
                            eng = nc.gpsimd
                        elif rb % 2 == 1:
                            eng = nc.sync
                        else:
                            eng = nc.scalar
                        eng.dma_start(xt_sb[:], xt_view[:, :, c0:c0 + 512])

                    q_ps = ps1.tile([128, 512], F32, tag="q")
                    qr_ps = ps1.tile([128, 512], F32, tag="qr")
                    k_ps = ps1.tile([128, 512], F32, tag="k")
                    kr_ps = ps1.tile([128, 512], F32, tag="kr")
                    v_ps = ps1.tile([128, 512], F32, tag="v")
                    for kt in range(KT):
                        st, sp = kt == 0, kt == KT - 1
                        for ps_t, w_t in [(q_ps, wq_sb), (qr_ps, wqr_sb),
                                          (k_ps, wk_sb), (kr_ps, wkr_sb)]:
                            nc.tensor.matmul(ps_t[:], w_t[:, kt, :],
                                             xt_sb[:, kt, :], start=st, stop=sp)
                        for vt in range(4):
                            nc.tensor.matmul(
                                v_ps[:, vt * 128:(vt + 1) * 128],
                                xt_sb[:, kt, vt * 128:(vt + 1) * 128],
                                wv_sb[:, kt, :], start=(st and vt == 0), stop=sp)

                    cc = c0 % N
                    tmp = p1.tile([128, 512], BF16, tag="rottmp")
                    for dst, a_ps, b_ps in [(qt_sb, q_ps, qr_ps),
                                            (kt_sb, k_ps, kr_ps)]:
                        dv = dst[:, c0:c0 + 512]
                        nc.vector.tensor_mul(dv, a_ps[:], cost_sb[:, cc:cc + 512])
                        nc.vector.tensor_mul(tmp[:], b_ps[:], sint_sb[:, cc:cc + 512])
                        nc.vector.tensor_add(dv, dv, tmp[:])

                    kt0 = rb * 4
                    va = vaug_sb[:].rearrange("p (t w) -> p t w", w=VAUGW)
                    vp = v_ps[:].rearrange("p (t c) -> p t c", c=128)
                    nc.vector.tensor_copy(va[:, kt0:kt0 + 4, 0:DH],
                                          vp[:, :, 0:DH])
                    nc.vector.tensor_copy(va[:, kt0:kt0 + 4, DH + 1:DH + 1 + DH],
                                          vp[:, :, DH:2 * DH])

            # wout + phase-3 constants: load behind phase-1 traffic
            nc.scalar.dma_start(wo_sb[:], wout_d.ap().rearrange(
                "(k p) n -> p k n", p=128))
            nc.sync.dma_start(boutb_sb[:], boutb_d[:, :])
            nc.sync.dma_start(selb_sb[:], selb_d[:, :])
            nc.sync.dma_start(selbi_sb[:], selbi_d[:, :])
            nc.sync.dma_start(sel16_sb[:], sel16_d[:, :])
            # zero the shard halves each batch's A2A never writes (the
            # receiving cores rely on them being exactly zero for the
            # split-phase-3 y0+y1 trick)
            zt = pp.tile([128, 512], BF16, tag="zt")
            nc.vector.memset(zt[:], 0.0)
            for b in range(B):
                for j in range(NCORES):
                    if (j // 4) != b:
                        r0 = j * SHARD_ROWS
                        nc.sync.dma_start(a2a_in[b][r0:r0 + 128, :], zt[:])
                        nc.sync.dma_start(a2a_in[b][r0 + 128:r0 + SHARD_ROWS, :],
                                          zt[0:2, :])

            # ---- Phase 2: attention, heads packed per sc tile,
            #      software-pipelined so PE never waits on ACT ----
            scale = float(DH ** -0.5)
            with tc.tile_pool(name="p2", bufs=3) as p2, \
                 tc.tile_pool(name="ps_sc", bufs=3, space="PSUM") as ps_sc, \
                 tc.tile_pool(name="ps_o", bufs=1, space="PSUM") as ps_o:
                for b in range(B):
                    for qc in range(NPASS):
                        qb = b * N + qc * QC
                        j = b * NPASS + qc
                        o_ps = [ps_o.tile([DH + 1, QC], F32, tag=f"o{h}",
                                          name=f"o{h}") for h in range(HPC)]
                        pt_prev = None
                        for kt in range(NKEYT + 1):
                            if kt < NKEYT:
                                g = b * NKEYT + kt
                                krow = b * N + kt * 128
                                sc = ps_sc.tile([128, 2 * QC], F32, tag="sc",
                                                name="sc")
                                for h in range(HPC):
                                    ho = h * DH
                                    nc.tensor.matmul(
                                        sc[:, h * QC:(h + 1) * QC],
                                        kt_sb[ho:ho + DH, krow:krow + 128],
                                        qt_sb[ho:ho + DH, qb:qb + QC],
                                        start=True, stop=True)
                                pt = p2.tile([128, 2 * QC], BF16, tag="p",
                                             name="pt")
                                nc.scalar.activation(
                                    pt[:], sc[:],
                                    mybir.ActivationFunctionType.Exp,
                                    bias=maskb_sb[:, g:g + 1], scale=scale)
                            if kt >= 1:
                                ktp = kt - 1
                                gp = b * NKEYT + ktp
                                for h in range(HPC):
                                    va_l = vaug_sb[:, gp * VAUGW + h * (DH + 1):
                                                   gp * VAUGW + (h + 1) * (DH + 1)]
                                    nc.tensor.matmul(
                                        o_ps[h][:], va_l,
                                        pt_prev[:, h * QC:(h + 1) * QC],
                                        start=(ktp == 0), stop=(ktp == NKEYT - 1))
                            pt_prev = pt

                        for h in range(HPC):
                            onb = p2.tile([DH + 1, QC], BF16, tag=f"onb{h}",
                                          name=f"onb{h}")
                            nc.vector.tensor_copy(onb[:], o_ps[h][:])
                            r0 = j * SHARD_ROWS + h * (DH + 1)
                            nc.sync.dma_start(a2a_in[b][r0:r0 + DH + 1, :],
                                              onb[:])
                    nc.gpsimd.collective_compute(
                        "AllToAll", mybir.AluOpType.bypass,
                        replica_groups=[list(range(NCORES))],
                        ins=[a2a_in[b].ap().opt()],
                        outs=[a2a_out[b].ap().opt()])

            # ---- Phase 3: per-batch blend/normalize/project.
            # batch 0 runs while AllToAll#1 is in flight. ----
            with tc.tile_pool(name="p3", bufs=1) as p3, \
                 tc.tile_pool(name="p3b", bufs=2) as p3b, \
                 tc.tile_pool(name="ps3", bufs=2, space="PSUM") as ps3, \
                 tc.tile_pool(name="psy", bufs=2, space="PSUM") as psy:
                for b in range(B):
                    av = a2a_out[b].ap().rearrange("(j q) n -> q j n",
                                                   q=SHARD_ROWS)
                    o_t = p3.tile([128, NCORES, 512], BF16, tag=f"oall{b}",
                                  name=f"oall{b}")
                    d_t = p3.tile([2 * NCORES, 512], BF16, tag=f"den{b}",
                                  name=f"den{b}")
                    nc.sync.dma_start(d_t[0:NCORES, :], av[DH:DH + 1, :, :])
                    nc.sync.dma_start(d_t[NCORES:2 * NCORES, :],
                                      av[CPC + 1:CPC + 2, :, :])
                    nc.sync.dma_start(o_t[0:DH, :, :], av[0:DH, :, :])
                    nc.scalar.dma_start(o_t[DH:CPC, :, :],
                                        av[DH + 1:CPC + 1, :, :])
                    # den + (1-sel): cores whose A2A shards were zeros get
                    # den=1 -> recip finite, o=0 -> y=0 there.
                    dadj = p3.tile([2 * NCORES, 512], F32, tag=f"dadj{b}",
                                   name=f"dadj{b}")
                    sel_ap = (selbi_sb if b == 0 else selb_sb)[0:2 * NCORES, :]
                    nc.vector.tensor_scalar_add(dadj[:], d_t[:], sel_ap)
                    recip = p3.tile([2 * NCORES, 512], F32, tag=f"recip{b}",
                                    name=f"recip{b}")
                    nc.vector.reciprocal_approx_fast(recip[:], dadj[:])
                    recipb = p3.tile([2 * NCORES, 512], BF16, tag=f"recipb{b}",
                                     name=f"recipb{b}")
                    nc.vector.tensor_copy(recipb[:], recip[:])

                    onorm = p3.tile([128, NCORES, 512], BF16, tag=f"onorm{b}",
                                    name=f"onorm{b}")
                    for j in range(NCORES):
                        div_ps = ps3.tile([128, 512], F32, tag="div",
                                          name="div")
                        nc.tensor.matmul(div_ps[:],
                                         sel16_sb[:, j * 128:(j + 1) * 128],
                                         recipb[:], start=True, stop=True)
                        nc.vector.tensor_mul(onorm[:, j, :], o_t[:, j, :],
                                             div_ps[:])

                    for rw in range(4):
                        y_ps = psy.tile([128, DM], F32, tag="y", name="y")
                        for j in range(NCORES):
                            st, sp = j == 0, j == NCORES - 1
                            for nb in range(2):
                                nc.tensor.matmul(
                                    y_ps[:, nb * 512:(nb + 1) * 512],
                                    onorm[:, j, rw * 128:(rw + 1) * 128],
                                    wo_sb[:, j, nb * 512:(nb + 1) * 512],
                                    start=st, stop=sp)
                        if b == 0:
                            # fold the output bias in here; y1 adds none
                            nc.vector.tensor_add(y0_sb[:, rw, :], y_ps[:],
                                                 boutb_sb[:])
                        else:
                            y_sb = p3b.tile([128, DM], F32, tag="y_sb")
                            nc.vector.tensor_add(y_sb[:], y_ps[:],
                                                 y0_sb[:, rw, :])
                            eng = nc.sync if rw % 2 == 0 else nc.scalar
                            eng.dma_start(out_d[rw * 128:(rw + 1) * 128, :],
                                          y_sb[:])

    nc.compile()
    return nc


_NC_CACHE = None


def kernel(x, mask, pos_emb, Wq, Wkv, Wout, bout):
    global LAST_EXEC_TIME_NS, LAST_TRACE_DIR, _NC_CACHE

    x = np.asarray(x, dtype=np.float32)
    mask = np.asarray(mask)
    pos_emb = np.asarray(pos_emb, dtype=np.float32)
    Wq = np.asarray(Wq, dtype=np.float32)
    Wkv = np.asarray(Wkv, dtype=np.float32)
    Wout = np.asarray(Wout, dtype=np.float32)
    bout = np.asarray(bout, dtype=np.float32)

    bf = ml_dtypes.bfloat16
    xt = np.ascontiguousarray(x.reshape(R, DM).T).astype(bf)
    wk_full = Wkv[:, :H * DH]
    wv_full = Wkv[:, H * DH:]
    cost = np.ascontiguousarray(np.tile(np.cos(pos_emb).T, (HPC, 1))).astype(bf)
    sint = np.ascontiguousarray(np.tile(np.sin(pos_emb).T, (HPC, 1))).astype(bf)
    maskb = np.ascontiguousarray(
        np.where(mask.reshape(R), 0.0, -1e5).astype(np.float32)
        .reshape(R // 128, 128).T)
    boutb = np.ascontiguousarray(
        np.broadcast_to(bout[None, :], (128, DM)).astype(np.float32))
    wqr = _rot_cols(Wq)
    wkr = _rot_cols(wk_full)
    # selector for broadcasting recip rows to the 128 inner channels of
    # core-slot j: rows 0:64 <- recip row j (head A), 64:128 <- row 8+j
    sel16 = np.zeros((16, 8 * 128), dtype=bf)
    for j in range(8):
        sel16[j, j * 128:j * 128 + DH] = 1.0
        sel16[8 + j, j * 128 + DH:j * 128 + 2 * DH] = 1.0

    in_maps = []
    for c in range(NCORES):
        cols = slice(c * CPC, (c + 1) * CPC)
        in_maps.append({
            "xt": xt,
            "wq": np.ascontiguousarray(Wq[:, cols]).astype(bf),
            "wqr": np.ascontiguousarray(wqr[:, cols]).astype(bf),
            "wk": np.ascontiguousarray(wk_full[:, cols]).astype(bf),
            "wkr": np.ascontiguousarray(wkr[:, cols]).astype(bf),
            "wv": np.ascontiguousarray(wv_full[:, cols]).astype(bf),
            "wout": Wout.astype(bf),
            "boutb": boutb,
            "cost": cost,
            "sint": sint,
            "maskb": maskb,
            "vones": np.ones((128, (R // 128) * 2), dtype=bf),
            "selb": np.full((128, 1), 1.0 if c < 4 else 0.0, dtype=np.float32),
            "selbi": np.full((128, 1), 0.0 if c < 4 else 1.0, dtype=np.float32),
            "sel16": sel16,
        })

    dbg = bool(int(os.environ.get("BASS_KERNEL_DEBUG", "0")))
    if _NC_CACHE is None:
        _NC_CACHE = build(dbg=dbg)
    nc = _NC_CACHE

    trace = bool(int(os.environ.get("BASS_KERNEL_TRACE", "0")))
    kwargs = {}
    if trace:
        _install_trace_shim()
        tdir = os.environ.get("BASS_TRACE_DIR", "/tmp/bass_trace_out")
        import shutil
        shutil.rmtree(tdir, ignore_errors=True)
        os.makedirs(tdir, exist_ok=True)
        kwargs["tmpdir"] = tdir
    res = bass_utils.run_bass_kernel_spmd(
        nc, in_maps, core_ids=list(range(NCORES)), trace=trace, **kwargs)
    LAST_EXEC_TIME_NS = res.exec_time_ns
    if res.instructions_and_trace is not None:
        LAST_TRACE_DIR = res.instructions_and_trace[1]
        globals()["LAST_INSTS"] = res.instructions_and_trace[0]

    globals()["LAST_RESULTS"] = res.results
    y = np.concatenate([res.results[c]["out"] for c in range(NCORES)], axis=0)
    return y.reshape(B, N, DM)


# revision 7
# speedup vs baseline: 1.5561x; 1.1591x over previous
"""Distributed multi-head attention kernel for 8 TRN2 NeuronCores.

Module: B=2, N=2048, D_MODEL=1024, H=16, D_HEAD=64 attention with
arbitrary rotary embedding, key-side boolean masking, softmax, and
output projection.

Sharding: head-parallel attention (2 heads per core, both batches),
one combined AllToAll (~1 MB/core, bf16, no padding) to switch to
row-parallel for the output projection. Each core returns a
[512, 1024] row block.

v3 design:
 - Attention software-pipelined per 512-q-row pass: both heads'
   [128,512] score blocks share one [128,1024] PSUM tile, one exp per
   key tile covers both heads, score MMs for kt+1 are emitted before
   attnV of kt so the PE never waits on the scalar engine.  PSUM:
   2x score (4 banks) + 2 o-accumulators + 2 recip-broadcast = 8.
 - Rotary on device: rot2(q) = ProtT.T @ q (constant +-1 permutation
   matmul) instead of host-rotated duplicate weight projections --
   halves the q/k projection matmul count.
 - Softmax denominators via a ones-column in V (lhsT = [v | 1], M=65);
   key mask folded into the exp as a per-partition bias.
 - Softmax normalization on the PRODUCING core each pass:
   reciprocal_approx_fast on the o accumulator (row 64 = den), one
   f32 broadcast matmul per head (ones[1,128] from partition 64),
   normalize numerators on DVE, ship normalized bf16 [64,512].
 - ONE AllToAll over [8*128, 512]: slot j = this core's pass
   j=(b*4+qc) output; received shard j = core j's heads for my rows.
   Phase 3 is then just 1 DMA + 64 projection matmuls + bias + out.
 - Dummy bridge matmuls keep the PE HAM clock warm while the
   collective is in flight so phase 3 runs at full clock.
"""
import os
import warnings

warnings.filterwarnings("ignore")
import numpy as np
import ml_dtypes

from concourse import bacc, tile, mybir, bass_utils

B, N, DM, H, DH = 2, 2048, 1024, 16, 64
R = B * N
NCORES = 8
HPC = 2
CPC = HPC * DH       # 128 chans per core
KT = 8               # contraction tiles over d_model
RB = 8               # row blocks of 512 over R
NKEYT = 16           # key tiles of 128 over N
ROWS_PER_CORE = R // NCORES  # 512
QC = 512             # q rows per attention pass
NPASS = N // QC      # 4 passes per batch

F32 = mybir.dt.float32
BF16 = mybir.dt.bfloat16

SHARD_ROWS = CPC          # 128: [hA 64 | hB 64] (normalized, no dens)
VAUGW = 2 * (DH + 1)      # 130 cols per key tile: [vA | 1 | vB | 1]
N_BRIDGE = 16             # paced bridge links spanning the collective wait

LAST_EXEC_TIME_NS = None
LAST_TRACE_DIR = None


def _install_trace_shim():
    import sys
    import types
    import ctypes
    import contextlib

    if "antenv.axon_hooks" in sys.modules:
        return
    so_path = "/opt/axon/libaxon_pjrt.so"
    hook = None
    if os.path.exists(so_path):
        lib = ctypes.CDLL(so_path)
        if hasattr(lib, "axon_start_nrt_profile"):
            lib.axon_start_nrt_profile.argtypes = [
                ctypes.POINTER(ctypes.c_int64), ctypes.c_size_t]
            lib.axon_start_nrt_profile.restype = ctypes.c_int64
            lib.axon_stop_nrt_profile.argtypes = [ctypes.c_char_p]
            lib.axon_stop_nrt_profile.restype = ctypes.c_int64

            @contextlib.contextmanager
            def _hook(output_dir, device_ids):
                import jax
                jax.devices()
                if device_ids:
                    ids = (ctypes.c_int64 * len(device_ids))(*device_ids)
                    rc = lib.axon_start_nrt_profile(ids, len(device_ids))
                else:
                    rc = lib.axon_start_nrt_profile(None, 0)
                if rc != 0:
                    raise RuntimeError(f"axon_start_nrt_profile rc={rc}")
                try:
                    yield
                finally:
                    n = lib.axon_stop_nrt_profile(str(output_dir).encode())
                    print(f"[trace] {n} profile file(s) -> {output_dir}")

            hook = _hook

    mod = types.ModuleType("antenv.axon_hooks")
    mod.get_axon_ntff_profile_hook = lambda: hook
    mod.set_axon_ntff_profile_hook = lambda h: None
    sys.modules["antenv.axon_hooks"] = mod
    bass_utils.upload_artifacts = lambda tmpdir: tmpdir


def build(dbg=False):
    nc = bacc.Bacc("TRN2", target_bir_lowering=False, debug=False,
                   num_devices=NCORES)

    xt_d = nc.dram_tensor("xt", [DM, R], BF16, kind="ExternalInput")
    wq_d = nc.dram_tensor("wq", [DM, CPC], BF16, kind="ExternalInput")
    wk_d = nc.dram_tensor("wk", [DM, CPC], BF16, kind="ExternalInput")
    wv_d = nc.dram_tensor("wv", [DM, CPC], BF16, kind="ExternalInput")
    prot_d = nc.dram_tensor("prot", [128, 128], BF16, kind="ExternalInput")
    wout_d = nc.dram_tensor("wout", [DM, DM], BF16, kind="ExternalInput")
    boutb_d = nc.dram_tensor("boutb", [128, DM], F32, kind="ExternalInput")
    cost_d = nc.dram_tensor("cost", [CPC, N], BF16, kind="ExternalInput")
    sint_d = nc.dram_tensor("sint", [CPC, N], BF16, kind="ExternalInput")
    maskb_d = nc.dram_tensor("maskb", [128, R // 128], F32, kind="ExternalInput")
    vones_d = nc.dram_tensor("vones", [128, (R // 128) * 2], BF16,
                             kind="ExternalInput")

    out_d = nc.dram_tensor("out", [ROWS_PER_CORE, DM], F32, kind="ExternalOutput")

    a2a_in = nc.dram_tensor("a2a_in", [NCORES * SHARD_ROWS, ROWS_PER_CORE],
                            BF16)
    a2a_out = nc.dram_tensor("a2a_out", [NCORES * SHARD_ROWS, ROWS_PER_CORE],
                             BF16)

    with tile.TileContext(nc) as tc:
        with tc.tile_pool(name="persist", bufs=1) as pp:
            wq_sb = pp.tile([128, KT, CPC], BF16, tag="wq")
            wk_sb = pp.tile([128, KT, CPC], BF16, tag="wk")
            wv_sb = pp.tile([128, KT, CPC], BF16, tag="wv")
            prot_sb = pp.tile([128, 128], BF16, tag="prot")
            cost_sb = pp.tile([CPC, N], BF16, tag="cost")
            sint_sb = pp.tile([CPC, N], BF16, tag="sint")
            maskb_sb = pp.tile([128, R // 128], F32, tag="maskb")
            boutb_sb = pp.tile([128, DM], F32, tag="boutb")
            qt_sb = pp.tile([CPC, R], BF16, tag="qt")
            kt_sb = pp.tile([CPC, R], BF16, tag="kt")
            vaug_sb = pp.tile([128, (R // 128) * VAUGW], BF16, tag="vaug")
            wo_sb = pp.tile([128, KT, DM], BF16, tag="wo")
            ones_sb = pp.tile([128, 128], F32, tag="ones")
            nc.vector.memset(ones_sb[:], 1.0)

            def ktview(d):
                return d.ap().rearrange("(k p) n -> p k n", p=128)

            xt_view = xt_d.ap().rearrange("(k p) n -> p k n", p=128)

            # first xt block + weights first so matmuls start early
            xt_sb0 = pp.tile([128, KT, 512], BF16, tag="xt0")
            for kt in range(KT):
                eng = nc.sync if kt % 2 == 0 else nc.scalar
                eng.dma_start(xt_sb0[:, kt, :], xt_view[:, kt, 0:512])
            nc.sync.dma_start(wq_sb[:], ktview(wq_d))
            nc.scalar.dma_start(wk_sb[:], ktview(wk_d))
            nc.sync.dma_start(wv_sb[:], ktview(wv_d))
            nc.scalar.dma_start(prot_sb[:], prot_d[:, :])
            # pre-load the ACT Exp table during the initial DMA wait
            warm_sb = pp.tile([1, 2], F32, tag="warm")
            nc.vector.memset(warm_sb[:], 0.0)
            nc.scalar.activation(warm_sb[0:1, 1:2], warm_sb[0:1, 0:1],
                                 mybir.ActivationFunctionType.Exp)
            nc.scalar.dma_start(cost_sb[:], cost_d[:, :])
            nc.sync.dma_start(sint_sb[:], sint_d[:, :])
            nc.scalar.dma_start(maskb_sb[:], maskb_d[:, :])
            ones_view = vaug_sb[:].rearrange("p (t u w) -> p (t u) w",
                                             u=2, w=DH + 1)[:, :, DH]
            nc.scalar.dma_start(ones_view, vones_d[:, :])

            # ---- Phase 1: projections + on-device rotary + v_aug ----
            with tc.tile_pool(name="p1", bufs=2) as p1, \
                 tc.tile_pool(name="ps1", bufs=1, space="PSUM") as ps1, \
                 tc.tile_pool(name="psr", bufs=2, space="PSUM") as psr:
                for rb in range(RB):
                    c0 = rb * 512
                    if rb == 0:
                        xt_sb = xt_sb0
                    else:
                        xt_sb = p1.tile([128, KT, 512], BF16, tag="xt")
                        if rb == 4:
                            eng = nc.gpsimd
                        elif rb % 2 == 1:
                            eng = nc.sync
                        else:
                            eng = nc.scalar
                        eng.dma_start(xt_sb[:], xt_view[:, :, c0:c0 + 512])

                    q_ps = ps1.tile([128, 512], F32, tag="q")
                    k_ps = ps1.tile([128, 512], F32, tag="k")
                    v_ps = ps1.tile([128, 512], F32, tag="v")
                    for kt in range(KT):
                        st, sp = kt == 0, kt == KT - 1
                        nc.tensor.matmul(q_ps[:], wq_sb[:, kt, :],
                                         xt_sb[:, kt, :], start=st, stop=sp)
                        nc.tensor.matmul(k_ps[:], wk_sb[:, kt, :],
                                         xt_sb[:, kt, :], start=st, stop=sp)
                        for vt in range(4):
                            nc.tensor.matmul(
                                v_ps[:, vt * 128:(vt + 1) * 128],
                                xt_sb[:, kt, vt * 128:(vt + 1) * 128],
                                wv_sb[:, kt, :], start=(st and vt == 0), stop=sp)

                    cc = c0 % N
                    for dst, a_ps, rtag in [(qt_sb, q_ps, "qraw"),
                                            (kt_sb, k_ps, "kraw")]:
                        raw = p1.tile([128, 512], BF16, tag=rtag)
                        nc.vector.tensor_copy(raw[:], a_ps[:])
                        rot_ps = psr.tile([128, 512], F32, tag="rot")
                        nc.tensor.matmul(rot_ps[:], prot_sb[:], raw[:],
                                         start=True, stop=True)
                        dv = dst[:, c0:c0 + 512]
                        tmp = p1.tile([128, 512], BF16, tag="rottmp")
                        nc.vector.tensor_mul(dv, raw[:], cost_sb[:, cc:cc + 512])
                        nc.vector.tensor_mul(tmp[:], rot_ps[:],
                                             sint_sb[:, cc:cc + 512])
                        nc.vector.tensor_add(dv, dv, tmp[:])

                    kt0 = rb * 4
                    va = vaug_sb[:].rearrange("p (t w) -> p t w", w=VAUGW)
                    vp = v_ps[:].rearrange("p (t c) -> p t c", c=128)
                    nc.vector.tensor_copy(va[:, kt0:kt0 + 4, 0:DH],
                                          vp[:, :, 0:DH])
                    nc.vector.tensor_copy(va[:, kt0:kt0 + 4, DH + 1:DH + 1 + DH],
                                          vp[:, :, DH:2 * DH])

            # wout + output bias: load behind phase-1 traffic
            nc.scalar.dma_start(wo_sb[:], wout_d.ap().rearrange(
                "(k p) n -> p k n", p=128))
            nc.sync.dma_start(boutb_sb[:], boutb_d[:, :])

            # ---- Phase 2: attention, heads packed per sc tile,
            #      software-pipelined, per-pass normalization ----
            scale = float(DH ** -0.5)
            with tc.tile_pool(name="p2", bufs=3) as p2, \
                 tc.tile_pool(name="ps_sc", bufs=2, space="PSUM") as ps_sc, \
                 tc.tile_pool(name="ps_o", bufs=1, space="PSUM") as ps_o, \
                 tc.tile_pool(name="ps_d", bufs=1, space="PSUM") as ps_d:
                for b in range(B):
                    for qc in range(NPASS):
                        qb = b * N + qc * QC
                        j = b * NPASS + qc
                        o_ps = [ps_o.tile([DH + 1, QC], F32, tag=f"o{h}",
                                          name=f"o{h}") for h in range(HPC)]
                        pt_prev = None
                        for kt in range(NKEYT + 1):
                            if kt < NKEYT:
                                g = b * NKEYT + kt
                                krow = b * N + kt * 128
                                sc = ps_sc.tile([128, 2 * QC], F32, tag="sc",
                                                name="sc")
                                for h in range(HPC):
                                    ho = h * DH
                                    nc.tensor.matmul(
                                        sc[:, h * QC:(h + 1) * QC],
                                        kt_sb[ho:ho + DH, krow:krow + 128],
                                        qt_sb[ho:ho + DH, qb:qb + QC],
                                        start=True, stop=True)
                                pt = p2.tile([128, 2 * QC], BF16, tag="p",
                                             name="pt")
                                nc.scalar.activation(
                                    pt[:], sc[:],
                                    mybir.ActivationFunctionType.Exp,
                                    bias=maskb_sb[:, g:g + 1], scale=scale)
                            if kt >= 1:
                                ktp = kt - 1
                                gp = b * NKEYT + ktp
                                for h in range(HPC):
                                    va_l = vaug_sb[:, gp * VAUGW + h * (DH + 1):
                                                   gp * VAUGW + (h + 1) * (DH + 1)]
                                    nc.tensor.matmul(
                                        o_ps[h][:], va_l,
                                        pt_prev[:, h * QC:(h + 1) * QC],
                                        start=(ktp == 0), stop=(ktp == NKEYT - 1))
                            pt_prev = pt

                        # per-pass normalization on the producing core:
                        # recip of the whole o tile (row 64 = den is all we
                        # use), broadcast row 64 via a K=1 f32 matmul, then
                        # scale the numerators and ship bf16.
                        for h in range(HPC):
                            rcp = p2.tile([DH + 1, QC], F32, tag=f"rcp{h}",
                                          name=f"rcp{h}")
                            nc.vector.reciprocal_approx_fast(rcp[:], o_ps[h][:])
                            div_ps = ps_d.tile([128, QC], F32, tag=f"div{h}",
                                               name=f"div{h}")
                            nc.tensor.matmul(div_ps[:], ones_sb[DH:DH + 1, :],
                                             rcp[DH:DH + 1, :],
                                             start=True, stop=True,
                                             tile_position=(64, 0))
                            div_sb = p2.tile([DH, QC], BF16, tag=f"dv{h}",
                                             name=f"dv{h}")
                            nc.vector.tensor_copy(div_sb[:], div_ps[0:DH, :])
                            onb = p2.tile([DH, QC], BF16, tag=f"onb{h}",
                                          name=f"onb{h}")
                            nc.vector.tensor_mul(onb[:], o_ps[h][0:DH, :],
                                                 div_sb[:])
                            r0 = j * SHARD_ROWS + h * DH
                            nc.sync.dma_start(a2a_in[r0:r0 + DH, :], onb[:])

                nc.gpsimd.collective_compute(
                    "AllToAll", mybir.AluOpType.bypass,
                    replica_groups=[list(range(NCORES))],
                    ins=[a2a_in.ap().opt()],
                    outs=[a2a_out.ap().opt()])

            # ---- Phase 3: gather + output projection ----
            with tc.tile_pool(name="p3", bufs=1) as p3, \
                 tc.tile_pool(name="p3b", bufs=2) as p3b, \
                 tc.tile_pool(name="psy", bufs=2, space="PSUM") as psy, \
                 tc.tile_pool(name="psbr", bufs=1, space="PSUM") as psbr:
                # bridge: keep the PE clock warm while the A2A flies.
                # A serial DVE copy chain paces 2 matmuls per ~2us link so
                # the PE sees activity in every HAM window without burning
                # through the budget early.  Sized just under the expected
                # collective wall time.
                br_ps = psbr.tile([128, 512], F32, tag="bridge")
                for i in range(N_BRIDGE):
                    dly = p3.tile([128, 2048], BF16, tag="dly")
                    nc.vector.tensor_copy(dly[:], cost_sb[:, 0:2048])
                    for u in range(2):
                        nc.tensor.matmul(br_ps[:], wq_sb[:, 0, :],
                                         dly[:, u * 512:(u + 1) * 512],
                                         start=(i == 0 and u == 0),
                                         stop=(i == N_BRIDGE - 1 and u == 1))

                av = a2a_out.ap().rearrange("(j p) n -> p j n", p=SHARD_ROWS)
                o_t = p3.tile([128, NCORES, 512], BF16, tag="oall")
                nc.sync.dma_start(o_t[0:DH, :, :], av[0:DH, :, :])
                nc.scalar.dma_start(o_t[DH:CPC, :, :], av[DH:CPC, :, :])

                for rw in range(4):
                    y_ps = psy.tile([128, DM], F32, tag="y", name="y")
                    for j in range(NCORES):
                        st, sp = j == 0, j == NCORES - 1
                        for nb in range(2):
                            nc.tensor.matmul(
                                y_ps[:, nb * 512:(nb + 1) * 512],
                                o_t[:, j, rw * 128:(rw + 1) * 128],
                                wo_sb[:, j, nb * 512:(nb + 1) * 512],
                                start=st, stop=sp)
                    y_sb = p3b.tile([128, DM], F32, tag="y_sb")
                    nc.vector.tensor_add(y_sb[:], y_ps[:], boutb_sb[:])
                    eng = nc.sync if rw % 2 == 0 else nc.scalar
                    eng.dma_start(out_d[rw * 128:(rw + 1) * 128, :], y_sb[:])

    nc.compile()
    return nc


_NC_CACHE = None


def kernel(x, mask, pos_emb, Wq, Wkv, Wout, bout):
    global LAST_EXEC_TIME_NS, LAST_TRACE_DIR, _NC_CACHE

    x = np.asarray(x, dtype=np.float32)
    mask = np.asarray(mask)
    pos_emb = np.asarray(pos_emb, dtype=np.float32)
    Wq = np.asarray(Wq, dtype=np.float32)
    Wkv = np.asarray(Wkv, dtype=np.float32)
    Wout = np.asarray(Wout, dtype=np.float32)
    bout = np.asarray(bout, dtype=np.float32)

    bf = ml_dtypes.bfloat16
    xt = np.ascontiguousarray(x.reshape(R, DM).T).astype(bf)
    wk_full = Wkv[:, :H * DH]
    wv_full = Wkv[:, H * DH:]
    cost = np.ascontiguousarray(np.tile(np.cos(pos_emb).T, (HPC, 1))).astype(bf)
    sint = np.ascontiguousarray(np.tile(np.sin(pos_emb).T, (HPC, 1))).astype(bf)
    maskb = np.ascontiguousarray(
        np.where(mask.reshape(R), 0.0, -1e5).astype(np.float32)
        .reshape(R // 128, 128).T)
    boutb = np.ascontiguousarray(
        np.broadcast_to(bout[None, :], (128, DM)).astype(np.float32))
    # rot2 as a matmul: rot2(q) = P @ q (q in [chan, row] layout);
    # lhsT for the tensor engine is P.T
    prot = np.zeros((128, 128), dtype=bf)
    for i in range(64):
        prot[2 * i + 1, 2 * i] = -1.0
        prot[2 * i, 2 * i + 1] = 1.0

    in_maps = []
    for c in range(NCORES):
        cols = slice(c * CPC, (c + 1) * CPC)
        in_maps.append({
            "xt": xt,
            "wq": np.ascontiguousarray(Wq[:, cols]).astype(bf),
            "wk": np.ascontiguousarray(wk_full[:, cols]).astype(bf),
            "wv": np.ascontiguousarray(wv_full[:, cols]).astype(bf),
            "prot": prot,
            "wout": Wout.astype(bf),
            "boutb": boutb,
            "cost": cost,
            "sint": sint,
            "maskb": maskb,
            "vones": np.ones((128, (R // 128) * 2), dtype=bf),
        })

    dbg = bool(int(os.environ.get("BASS_KERNEL_DEBUG", "0")))
    if _NC_CACHE is None:
        _NC_CACHE = build(dbg=dbg)
    nc = _NC_CACHE

    trace = bool(int(os.environ.get("BASS_KERNEL_TRACE", "0")))
    kwargs = {}
    if trace:
        _install_trace_shim()
        tdir = os.environ.get("BASS_TRACE_DIR", "/tmp/bass_trace_out")
        import shutil
        shutil.rmtree(tdir, ignore_errors=True)
        os.makedirs(tdir, exist_ok=True)
        kwargs["tmpdir"] = tdir
    res = bass_utils.run_bass_kernel_spmd(
        nc, in_maps, core_ids=list(range(NCORES)), trace=trace, **kwargs)
    LAST_EXEC_TIME_NS = res.exec_time_ns
    if res.instructions_and_trace is not None:
        LAST_TRACE_DIR = res.instructions_and_trace[1]
        globals()["LAST_INSTS"] = res.instructions_and_trace[0]

    globals()["LAST_RESULTS"] = res.results
    y = np.concatenate([res.results[c]["out"] for c in range(NCORES)], axis=0)
    return y.reshape(B, N, DM)
